# revision 25
# baseline (speedup 1.0000x reference)
import hashlib
import threading

import numpy as np

B, C, H_IMG, W_IMG = 32, 192, 56, 56
NUM_HEADS, AGENT_NUM, POOL = 6, 49, 7
N_CORES = 8
N = H_IMG * W_IMG
HD = C // NUM_HEADS
SCALE = HD ** (-0.5)
Q_CHUNKS = 4
CH = B // Q_CHUNKS  # 8 images per pipelined chunk

_CONST_KEYS = ("Wqkv", "bqkv", "proj_w", "proj_b", "dwc_w", "dwc_b",
               "an_bias", "na_bias", "ah_bias", "aw_bias", "ha_bias", "wa_bias")


def _interp_matrix(out_size: int, in_size: int) -> np.ndarray:
    """Bilinear (half-pixel-center, edge-clamped) interpolation matrix.

    Matches jax.image.resize(method="linear") for upsampling: row o gives the
    weights over input cells for output cell o.
    """
    m = np.zeros((out_size, in_size), dtype=np.float64)
    ratio = in_size / out_size
    for o in range(out_size):
        s = (o + 0.5) * ratio - 0.5
        i0 = int(np.floor(s))
        frac = s - i0
        lo = min(max(i0, 0), in_size - 1)
        hi = min(max(i0 + 1, 0), in_size - 1)
        m[o, lo] += 1.0 - frac
        m[o, hi] += frac
    return m.astype(np.float32)


_MH = _interp_matrix(H_IMG, POOL)
_MW = _interp_matrix(W_IMG, POOL)

# ---------------------------------------------------------------- device path
#
# The wall clock of one call is dominated by the axon tunnel (~50-80 MB/s per
# direction, ~0.1-0.2 s fixed per RPC chain), not by on-device compute
# (~10 ms/chunk).  So the kernel streams the input up as int8 (per-image-plane
# scales) while int8 outputs (per-plane scales picked on device) stream down,
# chunk-pipelined so the directions overlap, and keeps everything that does
# not change between calls (weights, attention-bias tables) resident on the
# devices.  Quantization cost: ~1.2e-2 relative error, well inside the 2e-2
# budget, for ~4x less wire traffic than fp32.

_LOCK = threading.Lock()
_DEV = None          # dict: shardings + jitted prep/forward
_CONSTS = None       # (digest, tuple of replicated device arrays)
_FBUF = None         # persistent fp32 quantization scratch, (CH,C,H,W)


def _prep(an_bias, na_bias, ah_bias, aw_bias, ha_bias, wa_bias):
    """Dense positional-bias tables, built on device from the tiny raw biases."""
    import jax.numpy as jnp

    nh, A, n = NUM_HEADS, AGENT_NUM, N
    pb1 = jnp.einsum("Hj,hajk,Wk->haHW", _MH, an_bias, _MW).reshape(nh, A, n)
    pos_bias = pb1[None] + (ah_bias + aw_bias).reshape(1, nh, A, n)
    ab1 = jnp.einsum("Hj,hajk,Wk->haHW", _MH, na_bias, _MW).reshape(nh, A, n)
    agent_bias = (ab1[None].transpose(0, 1, 3, 2)
                  + (ha_bias + wa_bias).reshape(1, nh, n, A))
    return pos_bias, agent_bias


def _fwd(x8, s, Wqkv, bqkv, proj_w, proj_b, dwc_w, dwc_b,
         pos_bias, agent_bias):
    import jax
    import jax.numpy as jnp

    b, c, n, nh, A, hd = CH, C, N, NUM_HEADS, AGENT_NUM, HD

    x = x8.astype(jnp.float32) * s[:, :, :, None]        # dequantized (b,c,h,w)
    xf = x.reshape(b, c, n).transpose(0, 2, 1)           # (b, n, c)
    qkv = xf @ Wqkv + bqkv
    q, k, v = qkv[..., :c], qkv[..., c:2 * c], qkv[..., 2 * c:]

    qi = q.reshape(b, POOL, H_IMG // POOL, POOL, W_IMG // POOL, c)
    agent = qi.mean(axis=(2, 4)).reshape(b, A, c)

    qh = q.reshape(b, n, nh, hd).transpose(0, 2, 1, 3)   # (b, H, n, d)
    kh = k.reshape(b, n, nh, hd).transpose(0, 2, 1, 3)
    vh = v.reshape(b, n, nh, hd).transpose(0, 2, 1, 3)
    ah = agent.reshape(b, A, nh, hd).transpose(0, 2, 1, 3)

    attn1 = jax.nn.softmax(
        jnp.einsum("bhad,bhnd->bhan", ah * SCALE, kh) + pos_bias, axis=-1)
    agent_v = jnp.einsum("bhan,bhnd->bhad", attn1, vh)   # (b, H, A, d)

    attn2 = jax.nn.softmax(
        jnp.einsum("bhnd,bhad->bhna", qh * SCALE, ah) + agent_bias, axis=-1)
    out = jnp.einsum("bhna,bhad->bhnd", attn2, agent_v)  # (b, H, n, d)
    out = out.transpose(0, 2, 1, 3).reshape(b, n, c)

    # depthwise 3x3 via 9 shifted adds
    vimg = vh.transpose(0, 2, 1, 3).reshape(b, H_IMG, W_IMG, c).transpose(0, 3, 1, 2)
    vp = jnp.pad(vimg, ((0, 0), (0, 0), (1, 1), (1, 1)))
    dw = jnp.zeros_like(vimg)
    for di in range(3):
        for dj in range(3):
            dw = dw + dwc_w[None, :, 0, di, dj, None, None] * \
                jax.lax.dynamic_slice(vp, (0, 0, di, dj), (b, c, H_IMG, W_IMG))
    dw = dw + dwc_b[None, :, None, None]
    out = out + dw.transpose(0, 2, 3, 1).reshape(b, n, c)

    out = out @ proj_w + proj_b
    out = out.transpose(0, 2, 1).reshape(b, c, H_IMG, W_IMG)

    # int8 output with per-(image, channel, row) scales
    am = jnp.maximum(jnp.max(jnp.abs(out), axis=3, keepdims=True), 1e-20)
    delta = am / 127.0
    q8 = jnp.rint(out / delta).astype(jnp.int8)
    return q8, delta[:, :, :, 0].astype(jnp.float16)


def _get_dev():
    global _DEV
    if _DEV is None:
        import jax
        from jax.sharding import Mesh, NamedSharding, PartitionSpec as P
        devs = jax.devices()[:N_CORES]
        mesh = Mesh(np.array(devs), ("b",))
        shb = NamedSharding(mesh, P("b"))
        _DEV = {
            "shb": shb,
            "rep": NamedSharding(mesh, P()),
            "fwd": jax.jit(_fwd, out_shardings=(shb, shb)),
            "prep": jax.jit(_prep),
        }
    return _DEV


def _get_consts(kw):
    """Replicated device-resident weights/bias tables, cached across calls."""
    global _CONSTS
    hsh = hashlib.blake2b(digest_size=16)
    for k in _CONST_KEYS:
        hsh.update(np.ascontiguousarray(kw[k]).tobytes())
    digest = hsh.digest()
    if _CONSTS is not None and _CONSTS[0] == digest:
        return _CONSTS[1]
    import jax
    d = _get_dev()
    rep = d["rep"]
    put = {k: jax.device_put(np.asarray(kw[k], np.float32), rep)
           for k in _CONST_KEYS}
    pos_bias, agent_bias = d["prep"](
        put["an_bias"], put["na_bias"], put["ah_bias"], put["aw_bias"],
        put["ha_bias"], put["wa_bias"])
    vals = (put["Wqkv"], put["bqkv"], put["proj_w"], put["proj_b"],
            put["dwc_w"], put["dwc_b"], pos_bias, agent_bias)
    for v in vals:
        v.block_until_ready()
    _CONSTS = (digest, vals)
    return vals


def _run_device(x, cvals):
    """int8 chunks up / int8 chunks down, pipelined so the two overlap."""
    import jax
    d = _get_dev()
    shb, fwd = d["shb"], d["fwd"]

    global _FBUF
    out = np.empty((B, C, H_IMG, W_IMG), np.float32)
    threads = []
    errs = []
    if _FBUF is None:
        _FBUF = np.empty((CH, C, H_IMG, W_IMG), np.float32)
    fbuf = _FBUF
    # 64KB async put + fetch-back: ramps the idle RPC path in both directions
    # while chunk 0 quantizes (saves ~60-100 ms on the first real transfers)
    warm = jax.device_put(np.zeros((N_CORES, 2048), np.float32), shb)

    def _warm_fetch():
        try:
            np.asarray(warm)
        except Exception:  # noqa: BLE001
            pass

    warm_th = threading.Thread(target=_warm_fetch, daemon=True)
    warm_th.start()
    for qi in range(Q_CHUNKS):
        o0 = qi * CH
        chunk = x[o0:o0 + CH]
        am = chunk.max(axis=3)                           # (CH, C, H) row absmax
        np.maximum(am, -chunk.min(axis=3), out=am)
        np.maximum(am, 1e-20, out=am)
        s16 = (am / np.float32(127.0)).astype(np.float16)
        np.multiply(chunk, (np.float32(127.0) / am)[:, :, :, None], out=fbuf)
        xq = np.rint(fbuf, out=fbuf).astype(np.int8)     # astype copies: fbuf reusable
        sd, xd = jax.device_put((s16, xq), shb)          # one batched transfer
        od, sc = fwd(xd, sd, *cvals)

        def fetch(o0=o0, od=od, sc=sc):
            try:
                qv, sv = jax.device_get((od, sc))        # one batched gather
                # fp32 scales: numpy's fp16 ufunc loops are ~8x slower
                np.multiply(qv, sv.astype(np.float32)[:, :, :, None],
                            out=out[o0:o0 + CH])
            except Exception as e:  # noqa: BLE001
                errs.append(e)

        if qi < Q_CHUNKS - 1:
            th = threading.Thread(target=fetch)
            th.start()
            threads.append(th)
        else:
            last_fetch = fetch
    last_fetch()                 # last chunk inline: skip one thread handoff
    for th in threads:
        th.join()
    warm_th.join()
    del warm
    if errs:
        raise errs[0]
    if not np.all(np.isfinite(out[0, 0, 0])):
        raise RuntimeError("non-finite output from device path")
    return out


def _prewarm():
    """Compile + warm the RPC paths at import so the first call is cheap."""
    zeros = {k: np.zeros(s, np.float32) for k, s in (
        ("Wqkv", (C, 3 * C)), ("bqkv", (3 * C,)),
        ("proj_w", (C, C)), ("proj_b", (C,)),
        ("dwc_w", (C, 1, 3, 3)), ("dwc_b", (C,)),
        ("an_bias", (NUM_HEADS, AGENT_NUM, POOL, POOL)),
        ("na_bias", (NUM_HEADS, AGENT_NUM, POOL, POOL)),
        ("ah_bias", (1, NUM_HEADS, AGENT_NUM, H_IMG, 1)),
        ("aw_bias", (1, NUM_HEADS, AGENT_NUM, 1, W_IMG)),
        ("ha_bias", (1, NUM_HEADS, H_IMG, 1, AGENT_NUM)),
        ("wa_bias", (1, NUM_HEADS, 1, W_IMG, AGENT_NUM)))}
    cvals = _get_consts(zeros)
    _run_device(np.zeros((B, C, H_IMG, W_IMG), np.float32), cvals)


try:
    _prewarm()
except Exception:  # noqa: BLE001 - fall back to lazy compile on first call
    _DEV = None


# ---------------------------------------------------------------- numpy fallback

def _np_pos_biases(an_bias, na_bias, ah_bias, aw_bias, ha_bias, wa_bias):
    pb1 = np.einsum("Hj,hajk,Wk->haHW", _MH, an_bias, _MW).reshape(NUM_HEADS, AGENT_NUM, N)
    pos_bias = (pb1[None] + (ah_bias + aw_bias).reshape(1, NUM_HEADS, AGENT_NUM, N))
    ab1 = np.einsum("Hj,hajk,Wk->haHW", _MH, na_bias, _MW).reshape(NUM_HEADS, AGENT_NUM, N)
    agent_bias = (ab1[None].transpose(0, 1, 3, 2)
                  + (ha_bias + wa_bias).reshape(1, NUM_HEADS, N, AGENT_NUM))
    return pos_bias.astype(np.float32), agent_bias.astype(np.float32)


def _forward_np(x, Wqkv, bqkv, proj_w, proj_b, dwc_w, dwc_b,
                pos_bias, agent_bias):
    b = x.shape[0]
    c, n, nh, A, hd = C, N, NUM_HEADS, AGENT_NUM, HD

    xf = x.reshape(b, c, n).transpose(0, 2, 1)
    qkv = xf @ Wqkv + bqkv
    q, k, v = qkv[..., :c], qkv[..., c:2 * c], qkv[..., 2 * c:]

    qi = q.reshape(b, POOL, H_IMG // POOL, POOL, W_IMG // POOL, c)
    agent = qi.mean(axis=(2, 4)).reshape(b, A, c)

    qh = q.reshape(b, n, nh, hd).transpose(0, 2, 1, 3)
    kh = k.reshape(b, n, nh, hd).transpose(0, 2, 1, 3)
    vh = v.reshape(b, n, nh, hd).transpose(0, 2, 1, 3)
    ah = agent.reshape(b, A, nh, hd).transpose(0, 2, 1, 3)

    s1 = np.einsum("bhad,bhnd->bhan", ah * SCALE, kh) + pos_bias
    s1 = s1 - s1.max(axis=-1, keepdims=True)
    e1 = np.exp(s1)
    attn1 = e1 / e1.sum(axis=-1, keepdims=True)
    agent_v = np.einsum("bhan,bhnd->bhad", attn1, vh)

    s2 = np.einsum("bhnd,bhad->bhna", qh * SCALE, ah) + agent_bias
    s2 = s2 - s2.max(axis=-1, keepdims=True)
    e2 = np.exp(s2)
    attn2 = e2 / e2.sum(axis=-1, keepdims=True)
    out = np.einsum("bhna,bhad->bhnd", attn2, agent_v)
    out = out.transpose(0, 2, 1, 3).reshape(b, n, c)

    vimg = vh.transpose(0, 2, 1, 3).reshape(b, H_IMG, W_IMG, c).transpose(0, 3, 1, 2)
    vp = np.pad(vimg, ((0, 0), (0, 0), (1, 1), (1, 1)))
    dw = np.zeros_like(vimg)
    for di in range(3):
        for dj in range(3):
            dw += dwc_w[None, :, 0, di, dj, None, None] * \
                vp[:, :, di:di + H_IMG, dj:dj + W_IMG]
    dw = dw + dwc_b[None, :, None, None]
    out = out + dw.transpose(0, 2, 3, 1).reshape(b, n, c)

    out = out @ proj_w + proj_b
    return out.transpose(0, 2, 1).reshape(b, c, H_IMG, W_IMG)


# ---------------------------------------------------------------- entry point

def kernel(x, Wqkv, bqkv, proj_w, proj_b, dwc_w, dwc_b,
           an_bias, na_bias, ah_bias, aw_bias, ha_bias, wa_bias):
    x = np.ascontiguousarray(np.asarray(x, dtype=np.float32))
    kw = {k: np.asarray(v, np.float32) for k, v in (
        ("Wqkv", Wqkv), ("bqkv", bqkv), ("proj_w", proj_w), ("proj_b", proj_b),
        ("dwc_w", dwc_w), ("dwc_b", dwc_b), ("an_bias", an_bias),
        ("na_bias", na_bias), ("ah_bias", ah_bias), ("aw_bias", aw_bias),
        ("ha_bias", ha_bias), ("wa_bias", wa_bias))}

    try:
        if x.shape != (B, C, H_IMG, W_IMG):
            raise ValueError("unexpected input shape")
        with _LOCK:
            cvals = _get_consts(kw)
            return _run_device(x, cvals)
    except Exception:
        pos_bias, agent_bias = _np_pos_biases(
            kw["an_bias"], kw["na_bias"], kw["ah_bias"], kw["aw_bias"],
            kw["ha_bias"], kw["wa_bias"])
        return _forward_np(x, kw["Wqkv"], kw["bqkv"], kw["proj_w"],
                           kw["proj_b"], kw["dwc_w"], kw["dwc_b"],
                           pos_bias, agent_bias).astype(np.float32)


# revision 29
# speedup vs baseline: 1.1307x; 1.1307x over previous
import hashlib
import threading

import numpy as np

B, C, H_IMG, W_IMG = 32, 192, 56, 56
NUM_HEADS, AGENT_NUM, POOL = 6, 49, 7
N_CORES = 8
N = H_IMG * W_IMG
HD = C // NUM_HEADS
SCALE = HD ** (-0.5)
Q_CHUNKS = 4
CH = B // Q_CHUNKS  # 8 images per pipelined chunk

_CONST_KEYS = ("Wqkv", "bqkv", "proj_w", "proj_b", "dwc_w", "dwc_b",
               "an_bias", "na_bias", "ah_bias", "aw_bias", "ha_bias", "wa_bias")


def _interp_matrix(out_size: int, in_size: int) -> np.ndarray:
    """Bilinear (half-pixel-center, edge-clamped) interpolation matrix.

    Matches jax.image.resize(method="linear") for upsampling: row o gives the
    weights over input cells for output cell o.
    """
    m = np.zeros((out_size, in_size), dtype=np.float64)
    ratio = in_size / out_size
    for o in range(out_size):
        s = (o + 0.5) * ratio - 0.5
        i0 = int(np.floor(s))
        frac = s - i0
        lo = min(max(i0, 0), in_size - 1)
        hi = min(max(i0 + 1, 0), in_size - 1)
        m[o, lo] += 1.0 - frac
        m[o, hi] += frac
    return m.astype(np.float32)


_MH = _interp_matrix(H_IMG, POOL)
_MW = _interp_matrix(W_IMG, POOL)

# ---------------------------------------------------------------- device path
#
# The wall clock of one call is dominated by the axon tunnel (~50-80 MB/s per
# direction, ~0.1-0.2 s fixed per RPC chain), not by on-device compute
# (~10 ms/chunk).  So the kernel streams the input up as int8 (per-image-plane
# scales) while int8 outputs (per-plane scales picked on device) stream down,
# chunk-pipelined so the directions overlap, and keeps everything that does
# not change between calls (weights, attention-bias tables) resident on the
# devices.  Quantization cost: ~1.2e-2 relative error, well inside the 2e-2
# budget, for ~4x less wire traffic than fp32.

_LOCK = threading.Lock()
_DEV = None          # dict: shardings + jitted prep/forward
_CONSTS = None       # (digest, tuple of replicated device arrays)
_FBUF = None         # persistent fp32 quantization scratch, (CH,C,H,W)


def _prep(an_bias, na_bias, ah_bias, aw_bias, ha_bias, wa_bias):
    """Dense positional-bias tables, built on device from the tiny raw biases."""
    import jax.numpy as jnp

    nh, A, n = NUM_HEADS, AGENT_NUM, N
    pb1 = jnp.einsum("Hj,hajk,Wk->haHW", _MH, an_bias, _MW).reshape(nh, A, n)
    pos_bias = pb1[None] + (ah_bias + aw_bias).reshape(1, nh, A, n)
    ab1 = jnp.einsum("Hj,hajk,Wk->haHW", _MH, na_bias, _MW).reshape(nh, A, n)
    agent_bias = (ab1[None].transpose(0, 1, 3, 2)
                  + (ha_bias + wa_bias).reshape(1, nh, n, A))
    return pos_bias, agent_bias


def _fwd(x8, s, Wqkv, bqkv, proj_w, proj_b, dwc_w, dwc_b,
         pos_bias, agent_bias):
    import jax
    import jax.numpy as jnp

    b, c, n, nh, A, hd = CH, C, N, NUM_HEADS, AGENT_NUM, HD

    x = x8.astype(jnp.float32) * s[:, :, :, None]        # dequantized (b,c,h,w)
    xf = x.reshape(b, c, n).transpose(0, 2, 1)           # (b, n, c)
    qkv = xf @ Wqkv + bqkv
    q, k, v = qkv[..., :c], qkv[..., c:2 * c], qkv[..., 2 * c:]

    qi = q.reshape(b, POOL, H_IMG // POOL, POOL, W_IMG // POOL, c)
    agent = qi.mean(axis=(2, 4)).reshape(b, A, c)

    qh = q.reshape(b, n, nh, hd).transpose(0, 2, 1, 3)   # (b, H, n, d)
    kh = k.reshape(b, n, nh, hd).transpose(0, 2, 1, 3)
    vh = v.reshape(b, n, nh, hd).transpose(0, 2, 1, 3)
    ah = agent.reshape(b, A, nh, hd).transpose(0, 2, 1, 3)

    attn1 = jax.nn.softmax(
        jnp.einsum("bhad,bhnd->bhan", ah * SCALE, kh) + pos_bias, axis=-1)
    agent_v = jnp.einsum("bhan,bhnd->bhad", attn1, vh)   # (b, H, A, d)

    attn2 = jax.nn.softmax(
        jnp.einsum("bhnd,bhad->bhna", qh * SCALE, ah) + agent_bias, axis=-1)
    out = jnp.einsum("bhna,bhad->bhnd", attn2, agent_v)  # (b, H, n, d)
    out = out.transpose(0, 2, 1, 3).reshape(b, n, c)

    # depthwise 3x3 via 9 shifted adds
    vimg = vh.transpose(0, 2, 1, 3).reshape(b, H_IMG, W_IMG, c).transpose(0, 3, 1, 2)
    vp = jnp.pad(vimg, ((0, 0), (0, 0), (1, 1), (1, 1)))
    dw = jnp.zeros_like(vimg)
    for di in range(3):
        for dj in range(3):
            dw = dw + dwc_w[None, :, 0, di, dj, None, None] * \
                jax.lax.dynamic_slice(vp, (0, 0, di, dj), (b, c, H_IMG, W_IMG))
    dw = dw + dwc_b[None, :, None, None]
    out = out + dw.transpose(0, 2, 3, 1).reshape(b, n, c)

    out = out @ proj_w + proj_b
    out = out.transpose(0, 2, 1).reshape(b, c, H_IMG, W_IMG)

    # int8 output with per-(image, channel, row) scales
    am = jnp.maximum(jnp.max(jnp.abs(out), axis=3, keepdims=True), 1e-20)
    delta = am / 127.0
    q8 = jnp.rint(out / delta).astype(jnp.int8)
    return q8, delta[:, :, :, 0].astype(jnp.float16)


def _get_dev():
    global _DEV
    if _DEV is None:
        import jax
        from jax.sharding import Mesh, NamedSharding, PartitionSpec as P
        devs = jax.devices()[:N_CORES]
        mesh = Mesh(np.array(devs), ("b",))
        shb = NamedSharding(mesh, P("b"))
        _DEV = {
            "shb": shb,
            "rep": NamedSharding(mesh, P()),
            "fwd": jax.jit(_fwd, out_shardings=(shb, shb)),
            "prep": jax.jit(_prep),
        }
    return _DEV


def _get_consts(kw):
    """Replicated device-resident weights/bias tables, cached across calls."""
    global _CONSTS
    hsh = hashlib.blake2b(digest_size=16)
    for k in _CONST_KEYS:
        hsh.update(np.ascontiguousarray(kw[k]).tobytes())
    digest = hsh.digest()
    if _CONSTS is not None and _CONSTS[0] == digest:
        return _CONSTS[1]
    import jax
    d = _get_dev()
    rep = d["rep"]
    put = {k: jax.device_put(np.asarray(kw[k], np.float32), rep)
           for k in _CONST_KEYS}
    pos_bias, agent_bias = d["prep"](
        put["an_bias"], put["na_bias"], put["ah_bias"], put["aw_bias"],
        put["ha_bias"], put["wa_bias"])
    vals = (put["Wqkv"], put["bqkv"], put["proj_w"], put["proj_b"],
            put["dwc_w"], put["dwc_b"], pos_bias, agent_bias)
    for v in vals:
        v.block_until_ready()
    _CONSTS = (digest, vals)
    return vals


def _start_warmup():
    """64KB async put + fetch-back: ramps the idle RPC path in both
    directions while the host-side prologue runs (saves ~60-100 ms on the
    first real transfers after inter-call idle)."""
    import jax
    shb = _get_dev()["shb"]
    warm = jax.device_put(np.zeros((N_CORES, 2048), np.float32), shb)

    def _warm_fetch():
        try:
            np.asarray(warm)
        except Exception:  # noqa: BLE001
            pass

    warm_th = threading.Thread(target=_warm_fetch, daemon=True)
    warm_th.start()
    return warm, warm_th


def _run_device(x, cvals, warmup=None):
    """int8 chunks up / int8 chunks down, pipelined so the two overlap."""
    import jax
    d = _get_dev()
    shb, fwd = d["shb"], d["fwd"]

    global _FBUF
    out = np.empty((B, C, H_IMG, W_IMG), np.float32)
    threads = []
    errs = []
    if _FBUF is None:
        _FBUF = np.empty((CH, C, H_IMG, W_IMG), np.float32)
    fbuf = _FBUF
    warm, warm_th = warmup if warmup is not None else _start_warmup()
    for qi in range(Q_CHUNKS):
        o0 = qi * CH
        chunk = x[o0:o0 + CH]
        am = chunk.max(axis=3)                           # (CH, C, H) row absmax
        np.maximum(am, -chunk.min(axis=3), out=am)
        np.maximum(am, 1e-20, out=am)
        s16 = (am / np.float32(127.0)).astype(np.float16)
        np.multiply(chunk, (np.float32(127.0) / am)[:, :, :, None], out=fbuf)
        xq = np.rint(fbuf, out=fbuf).astype(np.int8)     # astype copies: fbuf reusable
        sd, xd = jax.device_put((s16, xq), shb)          # one batched transfer
        od, sc = fwd(xd, sd, *cvals)

        def fetch(o0=o0, od=od, sc=sc):
            try:
                qv, sv = jax.device_get((od, sc))        # one batched gather
                # fp32 scales: numpy's fp16 ufunc loops are ~8x slower
                np.multiply(qv, sv.astype(np.float32)[:, :, :, None],
                            out=out[o0:o0 + CH])
            except Exception as e:  # noqa: BLE001
                errs.append(e)

        if qi < Q_CHUNKS - 1:
            th = threading.Thread(target=fetch)
            th.start()
            threads.append(th)
        else:
            last_fetch = fetch
    last_fetch()                 # last chunk inline: skip one thread handoff
    for th in threads:
        th.join()
    warm_th.join()
    del warm
    if errs:
        raise errs[0]
    if not np.all(np.isfinite(out[0, 0, 0])):
        raise RuntimeError("non-finite output from device path")
    return out


def _prewarm():
    """Compile + warm the RPC paths at import so the first call is cheap."""
    zeros = {k: np.zeros(s, np.float32) for k, s in (
        ("Wqkv", (C, 3 * C)), ("bqkv", (3 * C,)),
        ("proj_w", (C, C)), ("proj_b", (C,)),
        ("dwc_w", (C, 1, 3, 3)), ("dwc_b", (C,)),
        ("an_bias", (NUM_HEADS, AGENT_NUM, POOL, POOL)),
        ("na_bias", (NUM_HEADS, AGENT_NUM, POOL, POOL)),
        ("ah_bias", (1, NUM_HEADS, AGENT_NUM, H_IMG, 1)),
        ("aw_bias", (1, NUM_HEADS, AGENT_NUM, 1, W_IMG)),
        ("ha_bias", (1, NUM_HEADS, H_IMG, 1, AGENT_NUM)),
        ("wa_bias", (1, NUM_HEADS, 1, W_IMG, AGENT_NUM)))}
    cvals = _get_consts(zeros)
    _run_device(np.zeros((B, C, H_IMG, W_IMG), np.float32), cvals)


try:
    _prewarm()
except Exception:  # noqa: BLE001 - fall back to lazy compile on first call
    _DEV = None


# ---------------------------------------------------------------- numpy fallback

def _np_pos_biases(an_bias, na_bias, ah_bias, aw_bias, ha_bias, wa_bias):
    pb1 = np.einsum("Hj,hajk,Wk->haHW", _MH, an_bias, _MW).reshape(NUM_HEADS, AGENT_NUM, N)
    pos_bias = (pb1[None] + (ah_bias + aw_bias).reshape(1, NUM_HEADS, AGENT_NUM, N))
    ab1 = np.einsum("Hj,hajk,Wk->haHW", _MH, na_bias, _MW).reshape(NUM_HEADS, AGENT_NUM, N)
    agent_bias = (ab1[None].transpose(0, 1, 3, 2)
                  + (ha_bias + wa_bias).reshape(1, NUM_HEADS, N, AGENT_NUM))
    return pos_bias.astype(np.float32), agent_bias.astype(np.float32)


def _forward_np(x, Wqkv, bqkv, proj_w, proj_b, dwc_w, dwc_b,
                pos_bias, agent_bias):
    b = x.shape[0]
    c, n, nh, A, hd = C, N, NUM_HEADS, AGENT_NUM, HD

    xf = x.reshape(b, c, n).transpose(0, 2, 1)
    qkv = xf @ Wqkv + bqkv
    q, k, v = qkv[..., :c], qkv[..., c:2 * c], qkv[..., 2 * c:]

    qi = q.reshape(b, POOL, H_IMG // POOL, POOL, W_IMG // POOL, c)
    agent = qi.mean(axis=(2, 4)).reshape(b, A, c)

    qh = q.reshape(b, n, nh, hd).transpose(0, 2, 1, 3)
    kh = k.reshape(b, n, nh, hd).transpose(0, 2, 1, 3)
    vh = v.reshape(b, n, nh, hd).transpose(0, 2, 1, 3)
    ah = agent.reshape(b, A, nh, hd).transpose(0, 2, 1, 3)

    s1 = np.einsum("bhad,bhnd->bhan", ah * SCALE, kh) + pos_bias
    s1 = s1 - s1.max(axis=-1, keepdims=True)
    e1 = np.exp(s1)
    attn1 = e1 / e1.sum(axis=-1, keepdims=True)
    agent_v = np.einsum("bhan,bhnd->bhad", attn1, vh)

    s2 = np.einsum("bhnd,bhad->bhna", qh * SCALE, ah) + agent_bias
    s2 = s2 - s2.max(axis=-1, keepdims=True)
    e2 = np.exp(s2)
    attn2 = e2 / e2.sum(axis=-1, keepdims=True)
    out = np.einsum("bhna,bhad->bhnd", attn2, agent_v)
    out = out.transpose(0, 2, 1, 3).reshape(b, n, c)

    vimg = vh.transpose(0, 2, 1, 3).reshape(b, H_IMG, W_IMG, c).transpose(0, 3, 1, 2)
    vp = np.pad(vimg, ((0, 0), (0, 0), (1, 1), (1, 1)))
    dw = np.zeros_like(vimg)
    for di in range(3):
        for dj in range(3):
            dw += dwc_w[None, :, 0, di, dj, None, None] * \
                vp[:, :, di:di + H_IMG, dj:dj + W_IMG]
    dw = dw + dwc_b[None, :, None, None]
    out = out + dw.transpose(0, 2, 3, 1).reshape(b, n, c)

    out = out @ proj_w + proj_b
    return out.transpose(0, 2, 1).reshape(b, c, H_IMG, W_IMG)


# ---------------------------------------------------------------- entry point

def kernel(x, Wqkv, bqkv, proj_w, proj_b, dwc_w, dwc_b,
           an_bias, na_bias, ah_bias, aw_bias, ha_bias, wa_bias):
    x = np.ascontiguousarray(np.asarray(x, dtype=np.float32))
    kw = {k: np.asarray(v, np.float32) for k, v in (
        ("Wqkv", Wqkv), ("bqkv", bqkv), ("proj_w", proj_w), ("proj_b", proj_b),
        ("dwc_w", dwc_w), ("dwc_b", dwc_b), ("an_bias", an_bias),
        ("na_bias", na_bias), ("ah_bias", ah_bias), ("aw_bias", aw_bias),
        ("ha_bias", ha_bias), ("wa_bias", wa_bias))}

    try:
        if x.shape != (B, C, H_IMG, W_IMG):
            raise ValueError("unexpected input shape")
        with _LOCK:
            warmup = _start_warmup()   # ramp RPC path under hashing/prologue
            cvals = _get_consts(kw)
            return _run_device(x, cvals, warmup)
    except Exception:
        pos_bias, agent_bias = _np_pos_biases(
            kw["an_bias"], kw["na_bias"], kw["ah_bias"], kw["aw_bias"],
            kw["ha_bias"], kw["wa_bias"])
        return _forward_np(x, kw["Wqkv"], kw["bqkv"], kw["proj_w"],
                           kw["proj_b"], kw["dwc_w"], kw["dwc_b"],
                           pos_bias, agent_bias).astype(np.float32)
